# revision 24
# baseline (speedup 1.0000x reference)
"""Trainium2 Bass kernel for nn_C_Aggregation_24807731101830 — v3.

Key insight: the sequential Gauss-Seidel-like index-update scan is a FIXED
linear operator on the conv output (per channel, per batch).  Precompute
M [1156 x 1156] on host (exact fp64 linearization of the scan); then

    out[c, q'] = sum_q M[q', q] * conv[c, q] + bias[c]

since M row-sums are exactly 1 (each update is an average), the bias term
passes through unchanged.  M decays geometrically away from the diagonal
(factor 1/8 per in-row step, ~0.43 per row).

v10: column-packed banding + interleaved phases.  For each 128-row
interior-q block qb, the set of q' columns with max |MT[qb, q']| >= TAU
is a SINGLE contiguous run (~170-300 cols).  Stage 1 streams only those
runs (~1900 cols vs 22*128=2816 tile-granular).  PSUM bank reset
(start=True on first touch) zeroes all non-contributing columns, which
is exactly the correct value for border/bias columns.

Schedule per core (2 batches), all bf16 with fp32 PSUM accumulate:
  warmup(3) | s1(b0) | s2(b0) x s1(b1) interleaved | s2(b1)
Early input DMAs are split across both hwdge queues (SP+ACT) because a
DMA issue occupies the queue ~0.7us; wt streams per-cb during s2(b0);
s1(b1) sweeps sit between s2 cb-groups so their short matmuls' LDWEIGHTS
hide under 512-col streams.  Measured ~66us on 8 cores (PE-bound:
~100k cols at ~2.2-2.3 cols/ns sustained, 78.6TF/s peak = 2.4).
  stage1: tmp2[k, q'] = sum_q patches[q, k] * M^T[q, q']   (packed runs)
  stage2: out[c, q']  = sum_k w[c, k] * tmp2[k, q'] + bias  (dense)
Output q' in [0, 1122); host fills q' < 32 and >= 1122 with bias.
"""
import sys
import types
import numpy as np
import ml_dtypes

import concourse.mybir as mybir
from concourse import bass, tile
from concourse.bass_utils import run_bass_kernel_spmd
from contextlib import ExitStack

BF16 = mybir.dt.bfloat16
F32 = mybir.dt.float32
IDENT = mybir.ActivationFunctionType.Identity
np_bf16 = ml_dtypes.bfloat16

N_CORES = 8
B_LOC = 2          # batches per core
Q = 1156           # 34*34 flat grid
QP = 1152          # 9*128 q' blocks (q' >= 1122 handled on host)
QI = 1024          # interior q rows (zero-patch borders dropped)
NQB = QI // 128    # 8 contraction blocks
QLO = 32           # q' < 32 are all grid-row-0 borders (= bias)
QOUT = 1122        # end of shipped q' range; shipped width = QOUT-QLO
TAU = 3e-3
PN = 32

LAST_EXEC_NS = None


def _install_ntff_hook():
    try:
        import trn_agent_boot.trn_boot as tb
        mod = types.ModuleType("antenv.axon_hooks")
        holder = [None]
        mod.set_axon_ntff_profile_hook = lambda h: holder.__setitem__(0, h)
        mod.get_axon_ntff_profile_hook = lambda: holder[0]
        sys.modules["antenv.axon_hooks"] = mod
        import antenv
        antenv.axon_hooks = mod
        mod.set_axon_ntff_profile_hook(
            tb._ntff_profile_via_ctypes('/opt/axon/libaxon_pjrt.so'))
        return True
    except Exception:
        return False


def _split_sp_multiwaits(nc):
    """walrus for gen3 rejects >1 sync-wait on several instruction structs
    (TPB_CTRL, S3_LW, ...); hoist extra waits onto single-wait NOPs placed
    just before, on the same engine queue (semantically equivalent)."""
    cnt = 0
    for f in nc.m.functions:
        for blk in f.blocks:
            insts = blk.instructions
            i = 0
            while i < len(insts):
                inst = insts[i]
                si = getattr(inst, 'sync_info', None)
                if (getattr(inst, 'engine', None) is not None
                        and si is not None and si.on_wait and len(si.on_wait) > 1):
                    waits = list(si.on_wait)
                    new = []
                    for w in waits[:-1]:
                        nop = mybir.InstNoOp(name=f"mwfix-{inst.name}-{cnt}",
                                             ins=[], outs=[])
                        cnt += 1
                        nop.engine = inst.engine
                        nop.sync_info = mybir.SyncInfo(on_wait=[w], on_update=[])
                        new.append(nop)
                    inst.sync_info = mybir.SyncInfo(
                        on_wait=[waits[-1]], on_update=list(si.on_update or []))
                    insts[i:i] = new
                    i += len(new)
                i += 1
    return cnt


_M_CACHE = None


def _build_M():
    """Exact fp64 linearization of the reference's sequential scan."""
    global _M_CACHE
    if _M_CACHE is not None:
        return _M_CACHE
    M = np.eye(Q, dtype=np.float64)
    for i in range(1, PN - 1):
        for j in range(1, PN):
            idx = i * PN + j
            M[idx, :] = (M[idx - PN] + M[idx + PN] + M[idx - 1] + M[idx + 1]
                         + M[idx - PN - 1] + M[idx - PN + 1]
                         + M[idx + PN - 1] + M[idx + PN + 1]) / 8.0
    _M_CACHE = M
    return M


def _col_runs(MT):
    """Per qb: contiguous run [lo, hi) of q' columns with any
    |MT[qb-block, q']| >= TAU inside the shipped range (gaps <= 64 merged;
    with TAU=2e-3 each block is a single run)."""
    runs = []
    for qb in range(NQB):
        m = np.abs(MT[128 * qb:128 * (qb + 1), :]).max(axis=0) >= TAU
        m[:QLO] = False
        m[QOUT:] = False
        idx = np.where(m)[0]
        lo, hi = int(idx[0]), int(idx[-1]) + 1
        runs.append((lo, hi))
    # stage2 reads tmp2 over [QLO, QOUT); every column must be WRITTEN by
    # some matmul (uncovered psum columns are garbage, not zero).  The
    # extension columns are all-zero in MT so they just compute zeros.
    runs[0] = (min(runs[0][0], QLO), runs[0][1])
    runs[-1] = (runs[-1][0], max(runs[-1][1], QOUT))
    cover = QLO
    for lo, hi in runs:
        assert lo <= cover, (lo, cover)
        cover = max(cover, hi)
    assert cover >= QOUT
    return runs


def _build(runs):
    # packed column offsets per qb in the mt_packed tensor
    offs = []
    o = 0
    for (lo, hi) in runs:
        offs.append(o)
        o += hi - lo
    NCOL = o

    nc = bass.Bass("TRN2", target_bir_lowering=False)
    # all host-pretransposed: DMAs are flat contiguous copies
    pT_d = nc.declare_dram_parameter("pT", [B_LOC, 128, NQB * 768], BF16,
                                     isOutput=False)
    mt_d = nc.declare_dram_parameter("mt", [128, NCOL], BF16, isOutput=False)
    wT_d = nc.declare_dram_parameter("wT", [128, 6 * 768], BF16, isOutput=False)
    bias_d = nc.declare_dram_parameter("bias", [128, 6], F32, isOutput=False)
    xf_d = nc.declare_dram_parameter("xf", [B_LOC, 768, QOUT - QLO], BF16,
                                     isOutput=True)

    # stage-1 matmul segments: per qb, run split at psum regions
    # [0,512) [512,1024) (tile p1) and [1024,1152) (tile p1t)
    segs = []   # (qb, lo, hi, region)  region 0,1 = p1 banks, 2 = tail tile
    for qb, (lo, hi) in enumerate(runs):
        for r0, r1, reg in ((0, 512, 0), (512, 1024, 1), (1024, QP, 2)):
            a, bnd = max(lo, r0), min(hi, r1)
            if a < bnd:
                segs.append((qb, a, bnd, reg))

    with tile.TileContext(nc) as tc, ExitStack() as ctx:
        sb = ctx.enter_context(tc.tile_pool(name="sb", bufs=1))
        ps1 = ctx.enter_context(tc.tile_pool(name="ps1", bufs=2, space="PSUM"))
        ps2 = ctx.enter_context(tc.tile_pool(name="ps2", bufs=4, space="PSUM"))

        mt = sb.tile([128, NCOL], BF16, tag="mt")
        pt = sb.tile([128, B_LOC, 6, NQB, 128], BF16, tag="pt")
        pTr = pT_d.rearrange("b p (kb qb k) -> p b kb qb k", kb=6, qb=NQB)
        biast = sb.tile([128, 6], F32, tag="bias")
        # wt is cb-major: stage2's cb-th block needs only wt[:, cb] (0.2MB),
        # so later cb chunks stream in during stage2 itself.
        wt = sb.tile([128, 6, 6, 128], BF16, tag="wt")
        wTr = wT_d.rearrange("p (cb kb k) -> p cb kb k", cb=6, kb=6)
        # A DMA_DIRECT2D *issue* occupies the issuing engine ~0.7-0.9us, so
        # the early loads are split across BOTH hwdge queues (SP + ACT);
        # within each queue, strict FIFO in consumption order.  A short PE
        # warm-up on a zeroed tile covers the HAM ramp (~4us of busy to
        # reach k=8/8) while mt + pt(b0,k0) stream in, so real stage-1
        # matmuls start at full speed.
        warm = sb.tile([128, 512], BF16, tag="warm")
        nc.vector.memset(warm[:], 0.0)
        pw = ps2.tile([128, 512], F32, tag="p2", name="warmup")
        for _ in range(3):
            nc.tensor.matmul(pw[:], lhsT=warm[:, 0:128], rhs=warm[:],
                             start=True, stop=True)

        def load_pt(b, kb, eng=None):
            (eng or nc.sync).dma_start(pt[:, b, kb], pTr[:, b, kb])

        # SP carries mt (split so early sweeps start sooner), the odd pt
        # k-blocks, wt[cb0] and bias; ACT carries only k0/k2/k4 so the
        # ACT engine is free for stage-1 psum copies from ~12us on.
        # Per-queue FIFO transfer order == consumption order.
        nc.sync.dma_start(mt[:, 0:offs[2]], mt_d[:, 0:offs[2]])
        nc.scalar.dma_start(pt[:, 0, 0, 0:4], pTr[:, 0, 0, 0:4])
        nc.sync.dma_start(mt[:, offs[2]:offs[4]], mt_d[:, offs[2]:offs[4]])
        nc.scalar.dma_start(pt[:, 0, 0, 4:8], pTr[:, 0, 0, 4:8])
        nc.sync.dma_start(mt[:, offs[4]:NCOL], mt_d[:, offs[4]:NCOL])
        load_pt(0, 2, nc.scalar)
        load_pt(0, 1, nc.sync)
        load_pt(0, 4, nc.scalar)
        load_pt(0, 3, nc.sync)
        load_pt(0, 5, nc.sync)
        nc.sync.dma_start(wt[:, 0], wTr[:, 0])
        nc.sync.dma_start(biast[:], bias_d.rearrange("p c -> p c"))
        load_pt(1, 0, nc.sync)
        load_pt(1, 1, nc.sync)

        # separate tiles per batch so the Tile framework never serializes
        # batch-1 writes behind batch-0 reads via tile-level dependencies
        tmp2_ = [sb.tile([128, 6, QP], BF16, tag=f"tmp2_{b}",
                         name=f"tmp2_{b}") for b in range(B_LOC)]
        outS_ = [sb.tile([128, 6, QOUT - QLO], BF16, tag=f"outS_{b}",
                         name=f"outS_{b}") for b in range(B_LOC)]

        AOP = mybir.AluOpType
        # bank-aligned: chunk0 only needs the bank-0 copy of each sweep
        CH2 = [(QLO, 512 - QLO), (512, 512), (1024, QOUT - 1024)]
        ci = 0

        def sweep(b, kb):
            """tmp2[kb, q'] = sum_q patches[q, kb] * MT[q, q'] (packed runs)"""
            if True:
                p1 = ps1.tile([128, 1024], F32, tag="p1", name=f"p1_{b}_{kb}")
                p1t = ps2.tile([128, 128], F32, tag="p2", name=f"p1t_{b}_{kb}")
                seen = set()
                for (qb, lo, hi, reg) in segs:
                    dst = (p1[:, lo:hi] if reg < 2
                           else p1t[:, lo - 1024:hi - 1024])
                    off = offs[qb] + lo - runs[qb][0]
                    nc.tensor.matmul(
                        dst, lhsT=pt[:, b, kb, qb, :],
                        rhs=mt[:, off:off + hi - lo],
                        start=reg not in seen, stop=True,
                        skip_group_check=True)
                    seen.add(reg)
                # copy split across ACT & DVE so the 2-deep psum ring
                # never waits on a single engine's latency
                nc.vector.tensor_scalar_mul(tmp2_[b][:, kb, 0:512],
                                            p1[:, 0:512], 1.0)
                nc.scalar.mul(tmp2_[b][:, kb, 512:1024], p1[:, 512:1024], 1.0)
                nc.vector.tensor_scalar_mul(tmp2_[b][:, kb, 1024:QP],
                                            p1t[:], 1.0)

        def stage1(b):
            for kb in range(6):
                sweep(b, kb)

        def stage2(b, dma_cb=None, interleave=None):
            """out[c, q'] = sum_k w[c, k] * tmp2[k, q'] + bias"""
            nonlocal ci
            for cb in range(6):
                for (o, n) in CH2:
                    p2 = ps2.tile([128, n], F32, tag="p2",
                                  name=f"p2_{b}_{cb}_{o}")
                    for kb in range(6):
                        nc.tensor.matmul(
                            p2[:],
                            lhsT=wt[:, cb, kb, :],
                            rhs=tmp2_[b][:, kb, o:o + n],
                            start=(kb == 0), stop=(kb == 5))
                    # alternate copy engine so the psum ring never stalls
                    # the PE on a single engine's copy latency
                    oo = o - QLO
                    if ci % 2 == 0:
                        nc.scalar.activation(outS_[b][:, cb, oo:oo + n], p2[:],
                                             IDENT, bias=biast[:, cb:cb + 1])
                    else:
                        nc.vector.tensor_scalar(
                            outS_[b][:, cb, oo:oo + n], p2[:],
                            biast[:, cb:cb + 1], None, AOP.add)
                    ci += 1
                # one fully-contiguous 279KB DMA per (b,cb): strided
                # per-chunk DMAs cost ~0.7us issue each and 128-row
                # descriptor sets; the merged dst region is linear in dram
                dst = xf_d[b:b + 1, 128 * cb:128 * (cb + 1),
                           :].rearrange("b p q -> p (b q)")
                eng = nc.sync if cb % 2 == 0 else nc.scalar
                eng.dma_start(dst, outS_[b][:, cb, :])
                if dma_cb is not None:
                    dma_cb(cb)
                # interleaving batch-1 stage-1 sweeps between the cb groups
                # hides the small stage-1 matmuls' LDWEIGHTS under the long
                # 512-col stage-2 streams on either side
                if interleave is not None:
                    interleave(cb)

        def load_b1(cb):
            # deferred loads spread across stage2(b0): the next wt cb-chunk
            # and pT b1 k-blocks (k0/k1 were issued before stage2 started)
            if cb < 5:
                nc.sync.dma_start(wt[:, cb + 1], wTr[:, cb + 1])
            if cb < 4:
                load_pt(1, cb + 2)

        stage1(0)
        stage2(0, dma_cb=load_b1, interleave=lambda cb: sweep(1, cb))
        stage2(1)

    _split_sp_multiwaits(nc)
    return nc


_NC = None
_HOST = None


def _host_prep(w, b):
    """Input-independent host tensors: packed M columns, weights, bias."""
    global _HOST
    if _HOST is not None:
        return _HOST
    M = _build_M()
    qi = np.array([34 * (1 + i // 32) + 1 + i % 32 for i in range(QI)])
    MT = np.ascontiguousarray(M[:, qi].T)    # MT[q_int, q'] = M[q', qflat]
    runs = _col_runs(MT)
    NCOL = sum(hi - lo for lo, hi in runs)
    mt_host = np.empty((128, NCOL), dtype=np_bf16)
    o = 0
    for qb, (lo, hi) in enumerate(runs):
        mt_host[:, o:o + hi - lo] = MT[128 * qb:128 * (qb + 1),
                                       lo:hi].astype(np_bf16)
        o += hi - lo
    wm = np.asarray(w, dtype=np.float32).reshape(768, 768)   # [c, k]
    wT_host = np.ascontiguousarray(wm.T).astype(np_bf16)     # [k, c]
    # [k, c] -> [128(p), cb, kb, 128(c)] cb-major for streamed per-cb loads
    wT_host = np.ascontiguousarray(
        wT_host.reshape(6, 128, 6, 128).transpose(1, 2, 0, 3)
               .reshape(128, 6 * 768))
    bias_host = np.ascontiguousarray(
        np.asarray(b, dtype=np.float32).reshape(6, 128).T)   # [128, 6]
    _HOST = (runs, mt_host, wT_host, bias_host)
    return _HOST


def kernel(x: np.ndarray, w: np.ndarray, b: np.ndarray) -> np.ndarray:
    global _NC, LAST_EXEC_NS
    B, C, H, _ = x.shape          # 16, 3, 512, 512
    assert (B, C, H) == (16, 3, 512)

    runs, mt_host, wT_host, bias_host = _host_prep(w, b)

    # patches [b, q_int(1024), k] bf16, pre-transposed to [b, 128(q-in-block),
    # kb, qb, ks] so each per-(b,kb) DMA moves contiguous 2KB partition rows.
    xp = np.asarray(x, dtype=np.float32).reshape(B, 3, 32, 16, 32, 16)
    xp = xp.transpose(0, 2, 4, 1, 3, 5).reshape(B, QI, 768)     # [b, q_int, k]
    pT = np.ascontiguousarray(
        xp.astype(np_bf16).reshape(B, NQB, 128, 6, 128)
          .transpose(0, 2, 3, 1, 4).reshape(B, 128, 6 * QI // 128 * 128))

    if _NC is None:
        _NC = _build(runs)

    trace = _install_ntff_hook()
    in_maps = [{"pT": np.ascontiguousarray(pT[2 * r:2 * r + 2]),
                "mt": mt_host, "wT": wT_host, "bias": bias_host}
               for r in range(N_CORES)]
    import os
    tmpdir = os.environ.get("BASS_TMPDIR") or None
    try:
        res = run_bass_kernel_spmd(_NC, in_maps, core_ids=list(range(N_CORES)),
                                   trace=trace, tmpdir=tmpdir)
    except Exception:
        if not trace:
            raise
        res = run_bass_kernel_spmd(_NC, in_maps, core_ids=list(range(N_CORES)),
                                   trace=False, tmpdir=tmpdir)
    LAST_EXEC_NS = res.exec_time_ns
    globals()['LAST_RESULT'] = res

    xf = np.concatenate([res.results[r]["xf"] for r in range(N_CORES)], axis=0)
    xf_full = np.empty((B, 768, Q), dtype=np.float32)
    bias_col = np.asarray(b, np.float32)[None, :, None]
    xf_full[:, :, QLO:QOUT] = xf.astype(np.float32)
    xf_full[:, :, :QLO] = bias_col
    xf_full[:, :, QOUT:] = bias_col
    out = xf_full.reshape(B, 3, 544, 544)[:, :, 16:528, 16:528]
    return np.ascontiguousarray(out)


# revision 25
# speedup vs baseline: 1.0296x; 1.0296x over previous
"""Trainium2 Bass kernel for nn_C_Aggregation_24807731101830 — v3.

Key insight: the sequential Gauss-Seidel-like index-update scan is a FIXED
linear operator on the conv output (per channel, per batch).  Precompute
M [1156 x 1156] on host (exact fp64 linearization of the scan); then

    out[c, q'] = sum_q M[q', q] * conv[c, q] + bias[c]

since M row-sums are exactly 1 (each update is an average), the bias term
passes through unchanged.  M decays geometrically away from the diagonal
(factor 1/8 per in-row step, ~0.43 per row).

v10: column-packed banding + interleaved phases.  For each 128-row
interior-q block qb, the set of q' columns with max |MT[qb, q']| >= TAU
is a SINGLE contiguous run (~170-300 cols).  Stage 1 streams only those
runs (~1900 cols vs 22*128=2816 tile-granular).  PSUM bank reset
(start=True on first touch) zeroes all non-contributing columns, which
is exactly the correct value for border/bias columns.

Schedule per core (2 batches), all bf16 with fp32 PSUM accumulate:
  warmup(3) | s1(b0) | s2(b0) x s1(b1) interleaved | s2(b1)
Early input DMAs are split across both hwdge queues (SP+ACT) because a
DMA issue occupies the queue ~0.7us; wt streams per-cb during s2(b0);
s1(b1) sweeps sit between s2 cb-groups so their short matmuls' LDWEIGHTS
hide under 512-col streams.  Measured ~66us on 8 cores (PE-bound:
~100k cols at ~2.2-2.3 cols/ns sustained, 78.6TF/s peak = 2.4).
  stage1: tmp2[k, q'] = sum_q patches[q, k] * M^T[q, q']   (packed runs)
  stage2: out[c, q']  = sum_k w[c, k] * tmp2[k, q'] + bias  (dense)
Output q' in [0, 1122); host fills q' < 32 and >= 1122 with bias.
"""
import sys
import types
import numpy as np
import ml_dtypes

import concourse.mybir as mybir
from concourse import bass, tile
from concourse.bass_utils import run_bass_kernel_spmd
from contextlib import ExitStack

BF16 = mybir.dt.bfloat16
F32 = mybir.dt.float32
IDENT = mybir.ActivationFunctionType.Identity
np_bf16 = ml_dtypes.bfloat16

N_CORES = 8
B_LOC = 2          # batches per core
Q = 1156           # 34*34 flat grid
QP = 1152          # 9*128 q' blocks (q' >= 1122 handled on host)
QI = 1024          # interior q rows (zero-patch borders dropped)
NQB = QI // 128    # 8 contraction blocks
QLO = 32           # q' < 32 are all grid-row-0 borders (= bias)
QOUT = 1122        # end of shipped q' range; shipped width = QOUT-QLO
TAU = 3e-3
PN = 32

LAST_EXEC_NS = None


def _install_ntff_hook():
    try:
        import trn_agent_boot.trn_boot as tb
        mod = types.ModuleType("antenv.axon_hooks")
        holder = [None]
        mod.set_axon_ntff_profile_hook = lambda h: holder.__setitem__(0, h)
        mod.get_axon_ntff_profile_hook = lambda: holder[0]
        sys.modules["antenv.axon_hooks"] = mod
        import antenv
        antenv.axon_hooks = mod
        mod.set_axon_ntff_profile_hook(
            tb._ntff_profile_via_ctypes('/opt/axon/libaxon_pjrt.so'))
        return True
    except Exception:
        return False


def _split_sp_multiwaits(nc):
    """walrus for gen3 rejects >1 sync-wait on several instruction structs
    (TPB_CTRL, S3_LW, ...); hoist extra waits onto single-wait NOPs placed
    just before, on the same engine queue (semantically equivalent)."""
    cnt = 0
    for f in nc.m.functions:
        for blk in f.blocks:
            insts = blk.instructions
            i = 0
            while i < len(insts):
                inst = insts[i]
                si = getattr(inst, 'sync_info', None)
                if (getattr(inst, 'engine', None) is not None
                        and si is not None and si.on_wait and len(si.on_wait) > 1):
                    waits = list(si.on_wait)
                    new = []
                    for w in waits[:-1]:
                        nop = mybir.InstNoOp(name=f"mwfix-{inst.name}-{cnt}",
                                             ins=[], outs=[])
                        cnt += 1
                        nop.engine = inst.engine
                        nop.sync_info = mybir.SyncInfo(on_wait=[w], on_update=[])
                        new.append(nop)
                    inst.sync_info = mybir.SyncInfo(
                        on_wait=[waits[-1]], on_update=list(si.on_update or []))
                    insts[i:i] = new
                    i += len(new)
                i += 1
    return cnt


_M_CACHE = None


def _build_M():
    """Exact fp64 linearization of the reference's sequential scan."""
    global _M_CACHE
    if _M_CACHE is not None:
        return _M_CACHE
    M = np.eye(Q, dtype=np.float64)
    for i in range(1, PN - 1):
        for j in range(1, PN):
            idx = i * PN + j
            M[idx, :] = (M[idx - PN] + M[idx + PN] + M[idx - 1] + M[idx + 1]
                         + M[idx - PN - 1] + M[idx - PN + 1]
                         + M[idx + PN - 1] + M[idx + PN + 1]) / 8.0
    _M_CACHE = M
    return M


def _col_runs(MT):
    """Per qb: contiguous run [lo, hi) of q' columns with any
    |MT[qb-block, q']| >= TAU inside the shipped range (gaps <= 64 merged;
    with TAU=2e-3 each block is a single run)."""
    runs = []
    for qb in range(NQB):
        m = np.abs(MT[128 * qb:128 * (qb + 1), :]).max(axis=0) >= TAU
        m[:QLO] = False
        m[QOUT:] = False
        idx = np.where(m)[0]
        lo, hi = int(idx[0]), int(idx[-1]) + 1
        runs.append((lo, hi))
    # stage2 reads tmp2 over [QLO, QOUT); every column must be WRITTEN by
    # some matmul (uncovered psum columns are garbage, not zero).  The
    # extension columns are all-zero in MT so they just compute zeros.
    runs[0] = (min(runs[0][0], QLO), runs[0][1])
    runs[-1] = (runs[-1][0], max(runs[-1][1], QOUT))
    cover = QLO
    for lo, hi in runs:
        assert lo <= cover, (lo, cover)
        cover = max(cover, hi)
    assert cover >= QOUT
    return runs


def _build(runs):
    # packed column offsets per qb in the mt_packed tensor
    offs = []
    o = 0
    for (lo, hi) in runs:
        offs.append(o)
        o += hi - lo
    NCOL = o

    nc = bass.Bass("TRN2", target_bir_lowering=False)
    # all host-pretransposed: DMAs are flat contiguous copies
    pT_d = nc.declare_dram_parameter("pT", [B_LOC, 128, NQB * 768], BF16,
                                     isOutput=False)
    mt_d = nc.declare_dram_parameter("mt", [128, NCOL], BF16, isOutput=False)
    wT_d = nc.declare_dram_parameter("wT", [128, 6 * 768], BF16, isOutput=False)
    bias_d = nc.declare_dram_parameter("bias", [128, 6], F32, isOutput=False)
    xf_d = nc.declare_dram_parameter("xf", [B_LOC, 768, QOUT - QLO], BF16,
                                     isOutput=True)

    # stage-1 matmul segments: per qb, run split at psum regions
    # [0,512) [512,1024) (tile p1) and [1024,1152) (tile p1t)
    segs = []   # (qb, lo, hi, region)  region 0,1 = p1 banks, 2 = tail tile
    for qb, (lo, hi) in enumerate(runs):
        for r0, r1, reg in ((0, 512, 0), (512, 1024, 1), (1024, QP, 2)):
            a, bnd = max(lo, r0), min(hi, r1)
            if a < bnd:
                segs.append((qb, a, bnd, reg))

    with tile.TileContext(nc) as tc, ExitStack() as ctx:
        sb = ctx.enter_context(tc.tile_pool(name="sb", bufs=1))
        ps1 = ctx.enter_context(tc.tile_pool(name="ps1", bufs=2, space="PSUM"))
        ps2 = ctx.enter_context(tc.tile_pool(name="ps2", bufs=4, space="PSUM"))

        mt = sb.tile([128, NCOL], BF16, tag="mt")
        pt = sb.tile([128, B_LOC, 6, NQB, 128], BF16, tag="pt")
        pTr = pT_d.rearrange("b p (kb qb k) -> p b kb qb k", kb=6, qb=NQB)
        biast = sb.tile([128, 6], F32, tag="bias")
        # wt is cb-major: stage2's cb-th block needs only wt[:, cb] (0.2MB),
        # so later cb chunks stream in during stage2 itself.
        wt = sb.tile([128, 6, 6, 128], BF16, tag="wt")
        wTr = wT_d.rearrange("p (cb kb k) -> p cb kb k", cb=6, kb=6)
        # A DMA_DIRECT2D *issue* occupies the issuing engine ~0.7-0.9us, so
        # the early loads are split across BOTH hwdge queues (SP + ACT);
        # within each queue, strict FIFO in consumption order.  A short PE
        # warm-up on a zeroed tile covers the HAM ramp (~4us of busy to
        # reach k=8/8) while mt + pt(b0,k0) stream in, so real stage-1
        # matmuls start at full speed.
        warm = sb.tile([128, 512], BF16, tag="warm")
        nc.vector.memset(warm[:], 0.0)
        pw = ps2.tile([128, 512], F32, tag="p2", name="warmup")
        for _ in range(3):
            nc.tensor.matmul(pw[:], lhsT=warm[:, 0:128], rhs=warm[:],
                             start=True, stop=True)

        def load_pt(b, kb, eng=None):
            (eng or nc.sync).dma_start(pt[:, b, kb], pTr[:, b, kb])

        # SP carries mt (split so early sweeps start sooner), the odd pt
        # k-blocks, wt[cb0] and bias; ACT carries only k0/k2/k4 so the
        # ACT engine is free for stage-1 psum copies from ~12us on.
        # Per-queue FIFO transfer order == consumption order.
        nc.sync.dma_start(mt[:, 0:offs[2]], mt_d[:, 0:offs[2]])
        nc.scalar.dma_start(pt[:, 0, 0, 0:4], pTr[:, 0, 0, 0:4])
        nc.sync.dma_start(mt[:, offs[2]:offs[4]], mt_d[:, offs[2]:offs[4]])
        nc.scalar.dma_start(pt[:, 0, 0, 4:8], pTr[:, 0, 0, 4:8])
        nc.sync.dma_start(mt[:, offs[4]:NCOL], mt_d[:, offs[4]:NCOL])
        load_pt(0, 2, nc.scalar)
        load_pt(0, 1, nc.sync)
        load_pt(0, 4, nc.scalar)
        load_pt(0, 3, nc.sync)
        load_pt(0, 5, nc.sync)
        nc.sync.dma_start(wt[:, 0], wTr[:, 0])
        nc.sync.dma_start(biast[:], bias_d.rearrange("p c -> p c"))
        load_pt(1, 0, nc.sync)
        load_pt(1, 1, nc.sync)

        # separate tiles per batch so the Tile framework never serializes
        # batch-1 writes behind batch-0 reads via tile-level dependencies
        tmp2_ = [sb.tile([128, 6, QP], BF16, tag=f"tmp2_{b}",
                         name=f"tmp2_{b}") for b in range(B_LOC)]
        outS_ = [sb.tile([128, 6, QOUT - QLO], BF16, tag=f"outS_{b}",
                         name=f"outS_{b}") for b in range(B_LOC)]

        AOP = mybir.AluOpType
        # bank-aligned: chunk0 only needs the bank-0 copy of each sweep
        CH2 = [(QLO, 512 - QLO), (512, 512), (1024, QOUT - 1024)]
        ci = 0

        def sweep(b, kb):
            """tmp2[kb, q'] = sum_q patches[q, kb] * MT[q, q'] (packed runs)"""
            if True:
                p1 = ps1.tile([128, 1024], F32, tag="p1", name=f"p1_{b}_{kb}")
                p1t = ps2.tile([128, 128], F32, tag="p2", name=f"p1t_{b}_{kb}")
                seen = set()
                for (qb, lo, hi, reg) in segs:
                    dst = (p1[:, lo:hi] if reg < 2
                           else p1t[:, lo - 1024:hi - 1024])
                    off = offs[qb] + lo - runs[qb][0]
                    nc.tensor.matmul(
                        dst, lhsT=pt[:, b, kb, qb, :],
                        rhs=mt[:, off:off + hi - lo],
                        start=reg not in seen, stop=True,
                        skip_group_check=True)
                    seen.add(reg)
                # copy split across ACT & DVE so the 2-deep psum ring
                # never waits on a single engine's latency
                nc.vector.tensor_scalar_mul(tmp2_[b][:, kb, 0:512],
                                            p1[:, 0:512], 1.0)
                nc.scalar.mul(tmp2_[b][:, kb, 512:1024], p1[:, 512:1024], 1.0)
                nc.scalar.mul(tmp2_[b][:, kb, 1024:QP], p1t[:], 1.0)

        def stage1(b):
            for kb in range(6):
                sweep(b, kb)

        def stage2(b, dma_cb=None, interleave=None):
            """out[c, q'] = sum_k w[c, k] * tmp2[k, q'] + bias"""
            nonlocal ci
            for cb in range(6):
                for (o, n) in CH2:
                    p2 = ps2.tile([128, n], F32, tag="p2",
                                  name=f"p2_{b}_{cb}_{o}")
                    for kb in range(6):
                        nc.tensor.matmul(
                            p2[:],
                            lhsT=wt[:, cb, kb, :],
                            rhs=tmp2_[b][:, kb, o:o + n],
                            start=(kb == 0), stop=(kb == 5))
                    # alternate copy engine so the psum ring never stalls
                    # the PE on a single engine's copy latency
                    oo = o - QLO
                    if ci % 2 == 0:
                        nc.scalar.activation(outS_[b][:, cb, oo:oo + n], p2[:],
                                             IDENT, bias=biast[:, cb:cb + 1])
                    else:
                        nc.vector.tensor_scalar(
                            outS_[b][:, cb, oo:oo + n], p2[:],
                            biast[:, cb:cb + 1], None, AOP.add)
                    # ship each chunk as soon as it is copied: the
                    # final post-compute tail is then just the last chunk.
                    # Alternate the issuing queue so neither backs up.
                    dst = xf_d[b:b + 1, 128 * cb:128 * (cb + 1),
                               oo:oo + n].rearrange("b p q -> p (b q)")
                    eng = nc.sync if ci % 2 == 0 else nc.scalar
                    eng.dma_start(dst, outS_[b][:, cb, oo:oo + n])
                    ci += 1
                if dma_cb is not None:
                    dma_cb(cb)
                # interleaving batch-1 stage-1 sweeps between the cb groups
                # hides the small stage-1 matmuls' LDWEIGHTS under the long
                # 512-col stage-2 streams on either side
                if interleave is not None:
                    interleave(cb)

        def load_b1(cb):
            # deferred loads spread across stage2(b0): the next wt cb-chunk
            # and pT b1 k-blocks (k0/k1 were issued before stage2 started)
            if cb < 5:
                nc.sync.dma_start(wt[:, cb + 1], wTr[:, cb + 1])
            if cb < 4:
                load_pt(1, cb + 2)

        stage1(0)
        stage2(0, dma_cb=load_b1, interleave=lambda cb: sweep(1, cb))
        stage2(1)

    _split_sp_multiwaits(nc)
    return nc


_NC = None
_HOST = None


def _host_prep(w, b):
    """Input-independent host tensors: packed M columns, weights, bias."""
    global _HOST
    if _HOST is not None:
        return _HOST
    M = _build_M()
    qi = np.array([34 * (1 + i // 32) + 1 + i % 32 for i in range(QI)])
    MT = np.ascontiguousarray(M[:, qi].T)    # MT[q_int, q'] = M[q', qflat]
    runs = _col_runs(MT)
    NCOL = sum(hi - lo for lo, hi in runs)
    mt_host = np.empty((128, NCOL), dtype=np_bf16)
    o = 0
    for qb, (lo, hi) in enumerate(runs):
        mt_host[:, o:o + hi - lo] = MT[128 * qb:128 * (qb + 1),
                                       lo:hi].astype(np_bf16)
        o += hi - lo
    wm = np.asarray(w, dtype=np.float32).reshape(768, 768)   # [c, k]
    wT_host = np.ascontiguousarray(wm.T).astype(np_bf16)     # [k, c]
    # [k, c] -> [128(p), cb, kb, 128(c)] cb-major for streamed per-cb loads
    wT_host = np.ascontiguousarray(
        wT_host.reshape(6, 128, 6, 128).transpose(1, 2, 0, 3)
               .reshape(128, 6 * 768))
    bias_host = np.ascontiguousarray(
        np.asarray(b, dtype=np.float32).reshape(6, 128).T)   # [128, 6]
    _HOST = (runs, mt_host, wT_host, bias_host)
    return _HOST


def kernel(x: np.ndarray, w: np.ndarray, b: np.ndarray) -> np.ndarray:
    global _NC, LAST_EXEC_NS
    B, C, H, _ = x.shape          # 16, 3, 512, 512
    assert (B, C, H) == (16, 3, 512)

    runs, mt_host, wT_host, bias_host = _host_prep(w, b)

    # patches [b, q_int(1024), k] bf16, pre-transposed to [b, 128(q-in-block),
    # kb, qb, ks] so each per-(b,kb) DMA moves contiguous 2KB partition rows.
    xp = np.asarray(x, dtype=np.float32).reshape(B, 3, 32, 16, 32, 16)
    xp = xp.transpose(0, 2, 4, 1, 3, 5).reshape(B, QI, 768)     # [b, q_int, k]
    pT = np.ascontiguousarray(
        xp.astype(np_bf16).reshape(B, NQB, 128, 6, 128)
          .transpose(0, 2, 3, 1, 4).reshape(B, 128, 6 * QI // 128 * 128))

    if _NC is None:
        _NC = _build(runs)

    trace = _install_ntff_hook()
    in_maps = [{"pT": np.ascontiguousarray(pT[2 * r:2 * r + 2]),
                "mt": mt_host, "wT": wT_host, "bias": bias_host}
               for r in range(N_CORES)]
    import os
    tmpdir = os.environ.get("BASS_TMPDIR") or None
    try:
        res = run_bass_kernel_spmd(_NC, in_maps, core_ids=list(range(N_CORES)),
                                   trace=trace, tmpdir=tmpdir)
    except Exception:
        if not trace:
            raise
        res = run_bass_kernel_spmd(_NC, in_maps, core_ids=list(range(N_CORES)),
                                   trace=False, tmpdir=tmpdir)
    LAST_EXEC_NS = res.exec_time_ns
    globals()['LAST_RESULT'] = res

    xf = np.concatenate([res.results[r]["xf"] for r in range(N_CORES)], axis=0)
    xf_full = np.empty((B, 768, Q), dtype=np.float32)
    bias_col = np.asarray(b, np.float32)[None, :, None]
    xf_full[:, :, QLO:QOUT] = xf.astype(np.float32)
    xf_full[:, :, :QLO] = bias_col
    xf_full[:, :, QOUT:] = bias_col
    out = xf_full.reshape(B, 3, 544, 544)[:, :, 16:528, 16:528]
    return np.ascontiguousarray(out)
